# revision 27
# baseline (speedup 1.0000x reference)
"""Chamfer p=5 loss (nn_ChamferLossP) — Bass kernel for 8x TRN2 NeuronCores.

Sharding: data-parallel over the batch dim B=8, one batch per core; host
combines the per-core partial sums (the final "mean all-reduce").

Per-core device algorithm (direction 1 shown; direction 2 swaps x<->y):

  argmin_m ||x_n - y_m||^2  ==  argmax_m s[n,m],  s = 2 x.y - |y_m|^2.

  The PE materialises s in PSUM tiles [128n x 512m] with a single bf16
  matmul per tile: each fp32 factor is split into 3 bf16 limbs and the
  6 significant limb products per coordinate (plus 3 limbs of the -|y|^2
  term) form a 21-term contraction — fp32-accurate keys (~1e-7 rel) at
  bf16 speed (4x faster than the PE's multi-pass fp32 mode).

  Index extraction is two-level: the DVE reduces each PSUM tile to
  16-element group maxima (1 elem/cycle), then a short tensor_tensor_scan
  (running max, 2 cycles/elem but only N/16 elems) produces the prefix
  maxima r8 whose last column is the row max g.  The Scalar engine's
  Sign(g - r8) with accum_out counts groups strictly before the first
  attainment of g — the exact first-attainment group index (Sign(0)=0 on
  HW, probed).  One indirect DMA per 128-row chunk gathers that group's
  16 candidate points (48 contiguous floats); the epilogue recomputes the
  16 exact fp32 squared distances, picks the winner (first index on
  ties, matching jnp.argmin), and accumulates sum_c |x - nn|^5.
"""

import numpy as np
import ml_dtypes

import concourse.bass as bass
import concourse.bacc as bacc
import concourse.mybir as mybir
from concourse import bass_utils
from concourse.tile import TileContext

F32 = mybir.dt.float32
BF16 = mybir.dt.bfloat16
AF = mybir.ActivationFunctionType
ALU = mybir.AluOpType

B = 8
N_FULL = 4096
HALF_FULL = 2048
P = 128
R = 16              # argmin group size (candidates per gather)
KSPLIT = 21         # bf16 split-contraction terms
NEG_BIG = -3.0e38


def _build_nc(N=N_FULL, HALF=HALF_FULL, num_devices=B):
    NCH = N // P         # 128-row chunks per direction
    MMFD = min(512, HALF)
    NH = N // HALF       # psum tiles per chunk
    NG = N // R          # groups per row
    GH = HALF // R       # groups per psum tile

    nc = bacc.Bacc("TRN2", target_bir_lowering=False,
                   num_devices=num_devices)

    # augs columns: [x1_lhsT | y1_rhs | y2_lhsT | x2_rhs], each N wide, bf16.
    augs = nc.dram_tensor("augs", [KSPLIT, 4 * N], BF16,
                          kind="ExternalInput").ap()
    xr = nc.dram_tensor("xr", [N, 3], F32, kind="ExternalInput").ap()
    yr = nc.dram_tensor("yr", [N, 3], F32, kind="ExternalInput").ap()
    # consts row: [iota16 | iota16 + R]
    consts = nc.dram_tensor("consts", [P, 2 * R], F32,
                            kind="ExternalInput").ap()
    out_s = nc.dram_tensor("out_s", [P, 4], F32, kind="ExternalOutput").ap()

    with TileContext(nc) as tc:
        with (
            tc.tile_pool(name="const", bufs=1) as const_pool,
            tc.tile_pool(name="u", bufs=3) as u_pool,
            tc.tile_pool(name="r8", bufs=3) as r8_pool,
            tc.tile_pool(name="sgn", bufs=3) as sgn_pool,
            tc.tile_pool(name="idx", bufs=1) as idx_pool,
            tc.tile_pool(name="epi", bufs=1) as epi_pool,
            tc.tile_pool(name="psum", bufs=2, space="PSUM") as psum_pool,
        ):
            augs_sb = const_pool.tile([KSPLIT, 4 * N], BF16, tag="augs")
            # load order: dir-1 lhsT, then dir-1 rhs in quarters (so the
            # first matmuls start as soon as the first quarter lands),
            # then everything else.
            nc.sync.dma_start(augs_sb[:, 0:N], augs[:, 0:N])
            Q = N // 4
            for q in range(4):
                nc.sync.dma_start(augs_sb[:, N + q * Q:N + (q + 1) * Q],
                                  augs[:, N + q * Q:N + (q + 1) * Q])

            def aug(i):
                return augs_sb[:, i * N:(i + 1) * N]

            consts_sb = const_pool.tile([P, 2 * R], F32, tag="consts")
            nc.sync.dma_start(consts_sb[:], consts)

            dummy = const_pool.tile([P, 1], F32, tag="dummy")
            nc.vector.memset(dummy[:], 0.0)

            # epilogue "own point" tiles — load up front, they only
            # depend on the raw inputs
            ow_t = {}
            for dirn, own in ((1, xr), (2, yr)):
                ow = epi_pool.tile([P, NCH, 3], F32, tag=f"ow{dirn}",
                                   name=f"ow{dirn}")
                nc.sync.dma_start(
                    ow[:], own.rearrange("(c p) d -> p c d", p=P))
                ow_t[dirn] = ow

            for i in (2, 3):
                nc.sync.dma_start(augs_sb[:, i * N:(i + 1) * N],
                                  augs[:, i * N:(i + 1) * N])

            idxg_f = {1: idx_pool.tile([P, NCH], F32, tag="ig1", name="ig1"),
                      2: idx_pool.tile([P, NCH], F32, tag="ig2", name="ig2")}
            idxg_i = {1: idx_pool.tile([P, NCH], mybir.dt.int32, tag="ii1",
                                       name="ii1"),
                      2: idx_pool.tile([P, NCH], mybir.dt.int32, tag="ii2",
                                       name="ii2")}
            # gathered candidate groups, flat [P, NCH * R * 3]
            cand = {1: epi_pool.tile([P, NCH * R * 3], F32, tag="cand1",
                                     name="cand1"),
                    2: epi_pool.tile([P, NCH * R * 3], F32, tag="cand2",
                                     name="cand2")}

            partials = epi_pool.tile([P, 4], F32, tag="partials")
            FC = NCH * R * 3   # candidate floats per partition
            FK = NCH * R       # candidates per partition

            def epilogue(dirn, half, c0, c1):
                """Exact within-group argmin + sum |diff|^5 for chunk range
                [c0, c1) of a direction; writes partials column
                (dirn-1)*2 + half."""
                NC_h = c1 - c0
                FCh = NC_h * R * 3
                FKh = NC_h * R
                hh = f"{dirn}_{half}"
                cd = cand[dirn][:, c0 * R * 3:c1 * R * 3]
                ow = ow_t[dirn]
                owb = bass.AP(ow[:].tensor, ow[:].offset + c0 * 3,
                              [ow[:].ap[0], [3, NC_h], [0, R], [1, 3]])

                diff = epi_pool.tile([P, FCh], F32, tag=f"df{hh}",
                                     name=f"df{hh}")
                nc.vector.tensor_sub(
                    diff[:].rearrange("p (c k d) -> p c k d", k=R, d=3),
                    owb, cd.rearrange("p (c k d) -> p c k d", k=R, d=3))
                # sq first (feeds the DVE distance chain); abs afterwards
                # (only feeds the |.|^5 path, which has slack)
                sq = epi_pool.tile([P, FCh], F32, tag=f"sq{hh}",
                                   name=f"sq{hh}")
                nc.scalar.activation(out=sq[:], in_=diff[:], func=AF.Square,
                                     bias=0.0, scale=1.0)
                ad = epi_pool.tile([P, FCh], F32, tag=f"ab{hh}",
                                   name=f"ab{hh}")
                nc.scalar.activation(out=ad[:], in_=diff[:], func=AF.Abs,
                                     bias=0.0, scale=1.0)
                # start the p5 chain early — it runs on ACT/GpSimd and
                # must not trail the DVE selection chain
                q4 = epi_pool.tile([P, FCh], F32, tag=f"q4{hh}",
                                   name=f"q4{hh}")
                nc.scalar.activation(out=q4[:], in_=sq[:], func=AF.Square,
                                     bias=0.0, scale=1.0)
                p5e = epi_pool.tile([P, FCh], F32, tag=f"p5{hh}",
                                    name=f"p5{hh}")
                nc.gpsimd.tensor_mul(p5e[:], q4[:], ad[:])
                # squared L2 distance per candidate
                dd = epi_pool.tile([P, FKh], F32, tag=f"dd{hh}",
                                   name=f"dd{hh}")
                nc.vector.tensor_reduce(
                    out=dd[:], in_=sq[:].rearrange("p (k d) -> p k d", d=3),
                    axis=mybir.AxisListType.X, op=ALU.add)
                # min distance per row
                dmin = epi_pool.tile([P, NC_h], F32, tag=f"dm{hh}",
                                     name=f"dm{hh}")
                nc.vector.tensor_reduce(
                    out=dmin[:], in_=dd[:].rearrange("p (c k) -> p c k", k=R),
                    axis=mybir.AxisListType.X, op=ALU.min)
                dminb = bass.AP(dmin[:].tensor, dmin[:].offset,
                                [dmin[:].ap[0], [1, NC_h], [0, R]])
                mask = epi_pool.tile([P, FKh], F32, tag=f"mk{hh}",
                                     name=f"mk{hh}")
                nc.vector.tensor_tensor(
                    out=mask[:].rearrange("p (c k) -> p c k", k=R),
                    in0=dd[:].rearrange("p (c k) -> p c k", k=R),
                    in1=dminb, op=ALU.is_le)
                # first-attaining candidate: k* = min_k (iota_k + R*(1-mask))
                iotap = bass.AP(consts_sb[:].tensor, consts_sb[:].offset + R,
                               [consts_sb[:].ap[0], [0, NC_h], [1, R]])
                tkm = epi_pool.tile([P, FKh], F32, tag=f"tm{hh}",
                                    name=f"tm{hh}")
                nc.vector.tensor_scalar_mul(tkm[:], mask[:], -float(R))
                tk = epi_pool.tile([P, FKh], F32, tag=f"tk{hh}",
                                   name=f"tk{hh}")
                nc.vector.tensor_tensor(
                    out=tk[:].rearrange("p (c k) -> p c k", k=R),
                    in0=tkm[:].rearrange("p (c k) -> p c k", k=R),
                    in1=iotap, op=ALU.add)
                kstar = epi_pool.tile([P, NC_h], F32, tag=f"ks{hh}",
                                      name=f"ks{hh}")
                nc.vector.tensor_reduce(
                    out=kstar[:], in_=tk[:].rearrange("p (c k) -> p c k", k=R),
                    axis=mybir.AxisListType.X, op=ALU.min)
                # tk = (iota+R) - R*mask: attaining entries are exactly
                # iota_k, others iota_k+R, so min over k is k*.
                ksb = bass.AP(kstar[:].tensor, kstar[:].offset,
                              [kstar[:].ap[0], [1, NC_h], [0, R]])
                onehot = epi_pool.tile([P, FKh], F32, tag=f"oh{hh}",
                                       name=f"oh{hh}")
                nc.vector.tensor_tensor(
                    out=onehot[:].rearrange("p (c k) -> p c k", k=R),
                    in0=bass.AP(consts_sb[:].tensor, consts_sb[:].offset,
                                [consts_sb[:].ap[0], [0, NC_h], [1, R]]),
                    in1=ksb, op=ALU.is_equal)
                p5k = epi_pool.tile([P, FKh], F32, tag=f"pk{hh}",
                                    name=f"pk{hh}")
                nc.vector.tensor_reduce(
                    out=p5k[:], in_=p5e[:].rearrange("p (k d) -> p k d", d=3),
                    axis=mybir.AxisListType.X, op=ALU.add)
                psel = epi_pool.tile([P, FKh], F32, tag=f"pl{hh}",
                                     name=f"pl{hh}")
                nc.gpsimd.tensor_mul(psel[:], p5k[:], onehot[:])
                col = (dirn - 1) * 2 + half
                nc.vector.reduce_sum(partials[:, col:col + 1], psel[:],
                                     axis=mybir.AxisListType.X)

            for dirn in (1, 2):
                lhsT_all = aug(0) if dirn == 1 else aug(2)
                rhs_all = aug(1) if dirn == 1 else aug(3)
                gsrc = yr if dirn == 1 else xr
                gsrc_g = gsrc.rearrange("(g k) d -> g (k d)", k=R)
                for c in range(NCH):
                    r8 = r8_pool.tile([P, NG], F32, tag="r8")
                    u = u_pool.tile([P, NG], F32, tag="u")
                    for h in range(NH):
                        ps = psum_pool.tile([P, HALF], F32, tag="ps",
                                            space="PSUM")
                        for k in range(HALF // MMFD):
                            m0 = h * HALF + k * MMFD
                            nc.tensor.matmul(
                                ps[:, k * MMFD:(k + 1) * MMFD],
                                lhsT=lhsT_all[:, c * P:(c + 1) * P],
                                rhs=rhs_all[:, m0:m0 + MMFD],
                                start=True, stop=True,
                            )
                        # group maxima (R-wide) of this tile, 1 elem/cyc
                        nc.vector.tensor_reduce(
                            out=u[:, h * GH:(h + 1) * GH],
                            in_=ps[:].rearrange("p (g k) -> p g k", k=R),
                            axis=mybir.AxisListType.X,
                            op=ALU.max,
                        )
                    # prefix max over groups (2 cyc/elem, N/R elems)
                    nc.vector.tensor_tensor_scan(
                        out=r8[:],
                        data0=u[:],
                        data1=dummy[:, 0:1].to_broadcast([P, NG]),
                        initial=NEG_BIG,
                        op0=ALU.max,
                        op1=ALU.bypass,
                    )
                    # group index of first attainment of the row max
                    sgn = sgn_pool.tile([P, NG], BF16, tag="sgn")
                    nc.scalar.activation(
                        out=sgn[:], in_=r8[:, :],
                        func=AF.Sign,
                        bias=r8[:, NG - 1:NG],
                        scale=-1.0,
                        accum_out=idxg_f[dirn][:, c:c + 1],
                    )
                    nc.scalar.activation(
                        out=idxg_i[dirn][:, c:c + 1],
                        in_=idxg_f[dirn][:, c:c + 1],
                        func=AF.Copy, bias=0.0, scale=1.0)
                    # gather the 16-candidate group (48 contiguous floats)
                    nc.gpsimd.indirect_dma_start(
                        out=cand[dirn][:, c * R * 3:(c + 1) * R * 3],
                        out_offset=None,
                        in_=gsrc_g,
                        in_offset=bass.IndirectOffsetOnAxis(
                            ap=idxg_i[dirn][:, c:c + 1], axis=0),
                    )
                    # first-half epilogue overlaps the remaining chunks;
                    # only the second half tails the direction
                    if c == NCH // 2 - 1:
                        epilogue(dirn, 0, 0, NCH // 2)
                    elif c == NCH - 1:
                        epilogue(dirn, 1, NCH // 2, NCH)

            nc.sync.dma_start(out_s, partials[:])

    nc.compile()
    return nc


def _to_bf16(a):
    return a.astype(ml_dtypes.bfloat16)


def _split3(a):
    a = np.asarray(a, np.float32)
    h = _to_bf16(a)
    m = _to_bf16(a - h.astype(np.float32))
    l = _to_bf16(a - h.astype(np.float32) - m.astype(np.float32))
    return h, m, l


def _host_prep(xb, yb):
    xb = np.ascontiguousarray(xb, dtype=np.float32)
    yb = np.ascontiguousarray(yb, dtype=np.float32)
    n = xb.shape[0]
    ones = np.ones((n,), np.float32)

    def build(sta, mov, key_sq):
        """bf16 split terms for s = sum_c sta_c * (2 mov_c) - |mov|^2
        as seen with `sta` stationary; key_sq = -(|mov|^2)."""
        ta, tb = [], []
        for c in range(3):
            a, b = _split3(sta[:, c]), _split3(2.0 * mov[:, c])
            for i, j in ((0, 0), (0, 1), (0, 2), (1, 0), (1, 1), (2, 0)):
                ta.append(a[i])
                tb.append(b[j])
        sh, sm, sl = _split3(key_sq)
        ob = _to_bf16(ones)
        for s in (sh, sm, sl):
            ta.append(ob)
            tb.append(s)
        A = np.stack(ta).astype(ml_dtypes.bfloat16)
        Bm = np.stack(tb).astype(ml_dtypes.bfloat16)
        return A, Bm

    y2 = -(yb * yb).sum(-1)
    x2 = -(xb * xb).sum(-1)
    A1, B1 = build(xb, yb, y2)   # dir 1: lhsT = x terms, rhs = y terms
    A2, B2 = build(yb, xb, x2)   # dir 2: lhsT = y terms, rhs = x terms

    augs = np.empty((KSPLIT, 4 * n), ml_dtypes.bfloat16)
    augs[:, 0 * n:1 * n] = A1
    augs[:, 1 * n:2 * n] = B1
    augs[:, 2 * n:3 * n] = A2
    augs[:, 3 * n:4 * n] = B2

    iota = np.arange(R, dtype=np.float32)
    consts = np.tile(np.concatenate([iota, iota + R])[None, :], (P, 1))
    return {"augs": augs, "xr": xb, "yr": yb,
            "consts": np.ascontiguousarray(consts, np.float32)}


_NC = None


def _get_nc():
    global _NC
    if _NC is None:
        _NC = _build_nc()
    return _NC


def run_on_hw(x, y, **spmd_kwargs):
    """Run the SPMD kernel; returns (per-core out arrays, BassKernelResults)."""
    x = np.asarray(x, dtype=np.float32)
    y = np.asarray(y, dtype=np.float32)
    assert x.shape == (B, N_FULL, 3) and y.shape == (B, N_FULL, 3)
    nc = _get_nc()
    in_maps = [_host_prep(x[b], y[b]) for b in range(B)]
    res = bass_utils.run_bass_kernel_spmd(
        nc, in_maps, core_ids=list(range(B)), **spmd_kwargs)
    outs = [res.results[b]["out_s"] for b in range(B)]
    return outs, res


def kernel(x, y):
    outs, _ = run_on_hw(x, y)
    vals = []
    for o in outs:
        s = np.asarray(o, dtype=np.float64).sum(axis=0)  # [4] half-partials
        s1 = s[0] + s[1]
        s2 = s[2] + s[3]
        vals.append(s1 ** 0.2 + s2 ** 0.2)
    return np.float32(np.mean(vals))


# revision 28
# speedup vs baseline: 1.0037x; 1.0037x over previous
"""Chamfer p=5 loss (nn_ChamferLossP) — Bass kernel for 8x TRN2 NeuronCores.

Sharding: data-parallel over the batch dim B=8, one batch per core; host
combines the per-core partial sums (the final "mean all-reduce").

Per-core device algorithm (direction 1 shown; direction 2 swaps x<->y):

  argmin_m ||x_n - y_m||^2  ==  argmax_m s[n,m],  s = 2 x.y - |y_m|^2.

  The PE materialises s in PSUM tiles [128n x 512m] with a single bf16
  matmul per tile: each fp32 factor is split into 3 bf16 limbs and the
  6 significant limb products per coordinate (plus 3 limbs of the -|y|^2
  term) form a 21-term contraction — fp32-accurate keys (~1e-7 rel) at
  bf16 speed (4x faster than the PE's multi-pass fp32 mode).

  Index extraction is two-level: the DVE reduces each PSUM tile to
  16-element group maxima (1 elem/cycle), then a short tensor_tensor_scan
  (running max, 2 cycles/elem but only N/16 elems) produces the prefix
  maxima r8 whose last column is the row max g.  The Scalar engine's
  Sign(g - r8) with accum_out counts groups strictly before the first
  attainment of g — the exact first-attainment group index (Sign(0)=0 on
  HW, probed).  One indirect DMA per 128-row chunk gathers that group's
  16 candidate points (48 contiguous floats); the epilogue recomputes the
  16 exact fp32 squared distances, picks the winner (first index on
  ties, matching jnp.argmin), and accumulates sum_c |x - nn|^5.
"""

import numpy as np
import ml_dtypes

import concourse.bass as bass
import concourse.bacc as bacc
import concourse.mybir as mybir
from concourse import bass_utils
from concourse.tile import TileContext

F32 = mybir.dt.float32
BF16 = mybir.dt.bfloat16
AF = mybir.ActivationFunctionType
ALU = mybir.AluOpType

B = 8
N_FULL = 4096
HALF_FULL = 2048
P = 128
R = 16              # argmin group size (candidates per gather)
KSPLIT = 21         # bf16 split-contraction terms
NEG_BIG = -3.0e38


def _build_nc(N=N_FULL, HALF=HALF_FULL, num_devices=B):
    NCH = N // P         # 128-row chunks per direction
    MMFD = min(512, HALF)
    NH = N // HALF       # psum tiles per chunk
    NG = N // R          # groups per row
    GH = HALF // R       # groups per psum tile

    nc = bacc.Bacc("TRN2", target_bir_lowering=False,
                   num_devices=num_devices)

    # augs columns: [x1_lhsT | y1_rhs | y2_lhsT | x2_rhs], each N wide, bf16.
    augs = nc.dram_tensor("augs", [KSPLIT, 4 * N], BF16,
                          kind="ExternalInput").ap()
    xr = nc.dram_tensor("xr", [N, 3], F32, kind="ExternalInput").ap()
    yr = nc.dram_tensor("yr", [N, 3], F32, kind="ExternalInput").ap()
    # consts row: [iota16 | iota16 + R]
    consts = nc.dram_tensor("consts", [P, 2 * R], F32,
                            kind="ExternalInput").ap()
    out_s = nc.dram_tensor("out_s", [P, 4], F32, kind="ExternalOutput").ap()

    with TileContext(nc) as tc:
        with (
            tc.tile_pool(name="const", bufs=1) as const_pool,
            tc.tile_pool(name="u", bufs=3) as u_pool,
            tc.tile_pool(name="r8", bufs=3) as r8_pool,
            tc.tile_pool(name="sgn", bufs=3) as sgn_pool,
            tc.tile_pool(name="idx", bufs=1) as idx_pool,
            tc.tile_pool(name="epi", bufs=1) as epi_pool,
            tc.tile_pool(name="psum", bufs=2, space="PSUM") as psum_pool,
        ):
            augs_sb = const_pool.tile([KSPLIT, 4 * N], BF16, tag="augs")
            # load order: dir-1 lhsT, then dir-1 rhs in quarters (so the
            # first matmuls start as soon as the first quarter lands),
            # then everything else.
            # lhsT on the SP ring, rhs quarters on the ACT ring — the
            # two HWDGE rings transfer in parallel, halving time-to-first-MM
            nc.sync.dma_start(augs_sb[:, 0:N], augs[:, 0:N])
            Q = N // 4
            for q in range(4):
                nc.scalar.dma_start(augs_sb[:, N + q * Q:N + (q + 1) * Q],
                                    augs[:, N + q * Q:N + (q + 1) * Q])

            def aug(i):
                return augs_sb[:, i * N:(i + 1) * N]

            consts_sb = const_pool.tile([P, 2 * R], F32, tag="consts")
            nc.sync.dma_start(consts_sb[:], consts)

            dummy = const_pool.tile([P, 1], F32, tag="dummy")
            nc.vector.memset(dummy[:], 0.0)

            # epilogue "own point" tiles — load up front, they only
            # depend on the raw inputs
            ow_t = {}
            for dirn, own in ((1, xr), (2, yr)):
                ow = epi_pool.tile([P, NCH, 3], F32, tag=f"ow{dirn}",
                                   name=f"ow{dirn}")
                nc.sync.dma_start(
                    ow[:], own.rearrange("(c p) d -> p c d", p=P))
                ow_t[dirn] = ow

            for i in (2, 3):
                nc.sync.dma_start(augs_sb[:, i * N:(i + 1) * N],
                                  augs[:, i * N:(i + 1) * N])

            idxg_f = {1: idx_pool.tile([P, NCH], F32, tag="ig1", name="ig1"),
                      2: idx_pool.tile([P, NCH], F32, tag="ig2", name="ig2")}
            idxg_i = {1: idx_pool.tile([P, NCH], mybir.dt.int32, tag="ii1",
                                       name="ii1"),
                      2: idx_pool.tile([P, NCH], mybir.dt.int32, tag="ii2",
                                       name="ii2")}
            # gathered candidate groups, flat [P, NCH * R * 3]
            cand = {1: epi_pool.tile([P, NCH * R * 3], F32, tag="cand1",
                                     name="cand1"),
                    2: epi_pool.tile([P, NCH * R * 3], F32, tag="cand2",
                                     name="cand2")}

            partials = epi_pool.tile([P, 4], F32, tag="partials")
            FC = NCH * R * 3   # candidate floats per partition
            FK = NCH * R       # candidates per partition

            def epilogue(dirn, half, c0, c1):
                """Exact within-group argmin + sum |diff|^5 for chunk range
                [c0, c1) of a direction; writes partials column
                (dirn-1)*2 + half."""
                NC_h = c1 - c0
                FCh = NC_h * R * 3
                FKh = NC_h * R
                hh = f"{dirn}_{half}"
                cd = cand[dirn][:, c0 * R * 3:c1 * R * 3]
                ow = ow_t[dirn]
                owb = bass.AP(ow[:].tensor, ow[:].offset + c0 * 3,
                              [ow[:].ap[0], [3, NC_h], [0, R], [1, 3]])

                diff = epi_pool.tile([P, FCh], F32, tag=f"df{hh}",
                                     name=f"df{hh}")
                nc.vector.tensor_sub(
                    diff[:].rearrange("p (c k d) -> p c k d", k=R, d=3),
                    owb, cd.rearrange("p (c k d) -> p c k d", k=R, d=3))
                # sq first (feeds the DVE distance chain); abs afterwards
                # (only feeds the |.|^5 path, which has slack)
                sq = epi_pool.tile([P, FCh], F32, tag=f"sq{hh}",
                                   name=f"sq{hh}")
                nc.scalar.activation(out=sq[:], in_=diff[:], func=AF.Square,
                                     bias=0.0, scale=1.0)
                ad = epi_pool.tile([P, FCh], F32, tag=f"ab{hh}",
                                   name=f"ab{hh}")
                nc.scalar.activation(out=ad[:], in_=diff[:], func=AF.Abs,
                                     bias=0.0, scale=1.0)
                # start the p5 chain early — it runs on ACT/GpSimd and
                # must not trail the DVE selection chain
                q4 = epi_pool.tile([P, FCh], F32, tag=f"q4{hh}",
                                   name=f"q4{hh}")
                nc.scalar.activation(out=q4[:], in_=sq[:], func=AF.Square,
                                     bias=0.0, scale=1.0)
                p5e = epi_pool.tile([P, FCh], F32, tag=f"p5{hh}",
                                    name=f"p5{hh}")
                nc.gpsimd.tensor_mul(p5e[:], q4[:], ad[:])
                # squared L2 distance per candidate
                dd = epi_pool.tile([P, FKh], F32, tag=f"dd{hh}",
                                   name=f"dd{hh}")
                nc.vector.tensor_reduce(
                    out=dd[:], in_=sq[:].rearrange("p (k d) -> p k d", d=3),
                    axis=mybir.AxisListType.X, op=ALU.add)
                # min distance per row
                dmin = epi_pool.tile([P, NC_h], F32, tag=f"dm{hh}",
                                     name=f"dm{hh}")
                nc.vector.tensor_reduce(
                    out=dmin[:], in_=dd[:].rearrange("p (c k) -> p c k", k=R),
                    axis=mybir.AxisListType.X, op=ALU.min)
                dminb = bass.AP(dmin[:].tensor, dmin[:].offset,
                                [dmin[:].ap[0], [1, NC_h], [0, R]])
                mask = epi_pool.tile([P, FKh], F32, tag=f"mk{hh}",
                                     name=f"mk{hh}")
                nc.vector.tensor_tensor(
                    out=mask[:].rearrange("p (c k) -> p c k", k=R),
                    in0=dd[:].rearrange("p (c k) -> p c k", k=R),
                    in1=dminb, op=ALU.is_le)
                # first-attaining candidate: k* = min_k (iota_k + R*(1-mask))
                iotap = bass.AP(consts_sb[:].tensor, consts_sb[:].offset + R,
                               [consts_sb[:].ap[0], [0, NC_h], [1, R]])
                tkm = epi_pool.tile([P, FKh], F32, tag=f"tm{hh}",
                                    name=f"tm{hh}")
                nc.vector.tensor_scalar_mul(tkm[:], mask[:], -float(R))
                tk = epi_pool.tile([P, FKh], F32, tag=f"tk{hh}",
                                   name=f"tk{hh}")
                nc.vector.tensor_tensor(
                    out=tk[:].rearrange("p (c k) -> p c k", k=R),
                    in0=tkm[:].rearrange("p (c k) -> p c k", k=R),
                    in1=iotap, op=ALU.add)
                kstar = epi_pool.tile([P, NC_h], F32, tag=f"ks{hh}",
                                      name=f"ks{hh}")
                nc.vector.tensor_reduce(
                    out=kstar[:], in_=tk[:].rearrange("p (c k) -> p c k", k=R),
                    axis=mybir.AxisListType.X, op=ALU.min)
                # tk = (iota+R) - R*mask: attaining entries are exactly
                # iota_k, others iota_k+R, so min over k is k*.
                ksb = bass.AP(kstar[:].tensor, kstar[:].offset,
                              [kstar[:].ap[0], [1, NC_h], [0, R]])
                onehot = epi_pool.tile([P, FKh], F32, tag=f"oh{hh}",
                                       name=f"oh{hh}")
                nc.vector.tensor_tensor(
                    out=onehot[:].rearrange("p (c k) -> p c k", k=R),
                    in0=bass.AP(consts_sb[:].tensor, consts_sb[:].offset,
                                [consts_sb[:].ap[0], [0, NC_h], [1, R]]),
                    in1=ksb, op=ALU.is_equal)
                p5k = epi_pool.tile([P, FKh], F32, tag=f"pk{hh}",
                                    name=f"pk{hh}")
                nc.vector.tensor_reduce(
                    out=p5k[:], in_=p5e[:].rearrange("p (k d) -> p k d", d=3),
                    axis=mybir.AxisListType.X, op=ALU.add)
                psel = epi_pool.tile([P, FKh], F32, tag=f"pl{hh}",
                                     name=f"pl{hh}")
                nc.gpsimd.tensor_mul(psel[:], p5k[:], onehot[:])
                col = (dirn - 1) * 2 + half
                nc.vector.reduce_sum(partials[:, col:col + 1], psel[:],
                                     axis=mybir.AxisListType.X)

            for dirn in (1, 2):
                lhsT_all = aug(0) if dirn == 1 else aug(2)
                rhs_all = aug(1) if dirn == 1 else aug(3)
                gsrc = yr if dirn == 1 else xr
                gsrc_g = gsrc.rearrange("(g k) d -> g (k d)", k=R)
                for c in range(NCH):
                    r8 = r8_pool.tile([P, NG], F32, tag="r8")
                    u = u_pool.tile([P, NG], F32, tag="u")
                    for h in range(NH):
                        ps = psum_pool.tile([P, HALF], F32, tag="ps",
                                            space="PSUM")
                        for k in range(HALF // MMFD):
                            m0 = h * HALF + k * MMFD
                            nc.tensor.matmul(
                                ps[:, k * MMFD:(k + 1) * MMFD],
                                lhsT=lhsT_all[:, c * P:(c + 1) * P],
                                rhs=rhs_all[:, m0:m0 + MMFD],
                                start=True, stop=True,
                            )
                        # group maxima (R-wide) of this tile, 1 elem/cyc
                        nc.vector.tensor_reduce(
                            out=u[:, h * GH:(h + 1) * GH],
                            in_=ps[:].rearrange("p (g k) -> p g k", k=R),
                            axis=mybir.AxisListType.X,
                            op=ALU.max,
                        )
                    # prefix max over groups (2 cyc/elem, N/R elems)
                    nc.vector.tensor_tensor_scan(
                        out=r8[:],
                        data0=u[:],
                        data1=dummy[:, 0:1].to_broadcast([P, NG]),
                        initial=NEG_BIG,
                        op0=ALU.max,
                        op1=ALU.bypass,
                    )
                    # group index of first attainment of the row max
                    sgn = sgn_pool.tile([P, NG], BF16, tag="sgn")
                    nc.scalar.activation(
                        out=sgn[:], in_=r8[:, :],
                        func=AF.Sign,
                        bias=r8[:, NG - 1:NG],
                        scale=-1.0,
                        accum_out=idxg_f[dirn][:, c:c + 1],
                    )
                    nc.scalar.activation(
                        out=idxg_i[dirn][:, c:c + 1],
                        in_=idxg_f[dirn][:, c:c + 1],
                        func=AF.Copy, bias=0.0, scale=1.0)
                    # gather the 16-candidate group (48 contiguous floats)
                    nc.gpsimd.indirect_dma_start(
                        out=cand[dirn][:, c * R * 3:(c + 1) * R * 3],
                        out_offset=None,
                        in_=gsrc_g,
                        in_offset=bass.IndirectOffsetOnAxis(
                            ap=idxg_i[dirn][:, c:c + 1], axis=0),
                    )
                    # first-half epilogue overlaps the remaining chunks;
                    # only the second half tails the direction
                    if c == NCH // 2 + 1:
                        # two chunks after the half point: the ACT/DMA deps
                        # have finished by the time the DVE's strict-FIFO
                        # queue reaches these ops
                        epilogue(dirn, 0, 0, NCH // 2)
                    elif c == NCH - 1:
                        epilogue(dirn, 1, NCH // 2, NCH)

            nc.sync.dma_start(out_s, partials[:])

    nc.compile()
    return nc


def _to_bf16(a):
    return a.astype(ml_dtypes.bfloat16)


def _split3(a):
    a = np.asarray(a, np.float32)
    h = _to_bf16(a)
    m = _to_bf16(a - h.astype(np.float32))
    l = _to_bf16(a - h.astype(np.float32) - m.astype(np.float32))
    return h, m, l


def _host_prep(xb, yb):
    xb = np.ascontiguousarray(xb, dtype=np.float32)
    yb = np.ascontiguousarray(yb, dtype=np.float32)
    n = xb.shape[0]
    ones = np.ones((n,), np.float32)

    def build(sta, mov, key_sq):
        """bf16 split terms for s = sum_c sta_c * (2 mov_c) - |mov|^2
        as seen with `sta` stationary; key_sq = -(|mov|^2)."""
        ta, tb = [], []
        for c in range(3):
            a, b = _split3(sta[:, c]), _split3(2.0 * mov[:, c])
            for i, j in ((0, 0), (0, 1), (0, 2), (1, 0), (1, 1), (2, 0)):
                ta.append(a[i])
                tb.append(b[j])
        sh, sm, sl = _split3(key_sq)
        ob = _to_bf16(ones)
        for s in (sh, sm, sl):
            ta.append(ob)
            tb.append(s)
        A = np.stack(ta).astype(ml_dtypes.bfloat16)
        Bm = np.stack(tb).astype(ml_dtypes.bfloat16)
        return A, Bm

    y2 = -(yb * yb).sum(-1)
    x2 = -(xb * xb).sum(-1)
    A1, B1 = build(xb, yb, y2)   # dir 1: lhsT = x terms, rhs = y terms
    A2, B2 = build(yb, xb, x2)   # dir 2: lhsT = y terms, rhs = x terms

    augs = np.empty((KSPLIT, 4 * n), ml_dtypes.bfloat16)
    augs[:, 0 * n:1 * n] = A1
    augs[:, 1 * n:2 * n] = B1
    augs[:, 2 * n:3 * n] = A2
    augs[:, 3 * n:4 * n] = B2

    iota = np.arange(R, dtype=np.float32)
    consts = np.tile(np.concatenate([iota, iota + R])[None, :], (P, 1))
    return {"augs": augs, "xr": xb, "yr": yb,
            "consts": np.ascontiguousarray(consts, np.float32)}


_NC = None


def _get_nc():
    global _NC
    if _NC is None:
        _NC = _build_nc()
    return _NC


def run_on_hw(x, y, **spmd_kwargs):
    """Run the SPMD kernel; returns (per-core out arrays, BassKernelResults)."""
    x = np.asarray(x, dtype=np.float32)
    y = np.asarray(y, dtype=np.float32)
    assert x.shape == (B, N_FULL, 3) and y.shape == (B, N_FULL, 3)
    nc = _get_nc()
    in_maps = [_host_prep(x[b], y[b]) for b in range(B)]
    res = bass_utils.run_bass_kernel_spmd(
        nc, in_maps, core_ids=list(range(B)), **spmd_kwargs)
    outs = [res.results[b]["out_s"] for b in range(B)]
    return outs, res


def kernel(x, y):
    outs, _ = run_on_hw(x, y)
    vals = []
    for o in outs:
        s = np.asarray(o, dtype=np.float64).sum(axis=0)  # [4] half-partials
        s1 = s[0] + s[1]
        s2 = s[2] + s[3]
        vals.append(s1 ** 0.2 + s2 ** 0.2)
    return np.float32(np.mean(vals))


# revision 29
# speedup vs baseline: 1.0046x; 1.0009x over previous
"""Chamfer p=5 loss (nn_ChamferLossP) — Bass kernel for 8x TRN2 NeuronCores.

Sharding: data-parallel over the batch dim B=8, one batch per core; host
combines the per-core partial sums (the final "mean all-reduce").

Per-core device algorithm (direction 1 shown; direction 2 swaps x<->y):

  argmin_m ||x_n - y_m||^2  ==  argmax_m s[n,m],  s = 2 x.y - |y_m|^2.

  The PE materialises s in PSUM tiles [128n x 512m] with a single bf16
  matmul per tile: each fp32 factor is split into 3 bf16 limbs and the
  6 significant limb products per coordinate (plus 3 limbs of the -|y|^2
  term) form a 21-term contraction — fp32-accurate keys (~1e-7 rel) at
  bf16 speed (4x faster than the PE's multi-pass fp32 mode).

  Index extraction is two-level: the DVE reduces each PSUM tile to
  16-element group maxima (1 elem/cycle), then a short tensor_tensor_scan
  (running max, 2 cycles/elem but only N/16 elems) produces the prefix
  maxima r8 whose last column is the row max g.  The Scalar engine's
  Sign(g - r8) with accum_out counts groups strictly before the first
  attainment of g — the exact first-attainment group index (Sign(0)=0 on
  HW, probed).  One indirect DMA per 128-row chunk gathers that group's
  16 candidate points (48 contiguous floats); the epilogue recomputes the
  16 exact fp32 squared distances, picks the winner (first index on
  ties, matching jnp.argmin), and accumulates sum_c |x - nn|^5.
"""

import numpy as np
import ml_dtypes

import concourse.bass as bass
import concourse.bacc as bacc
import concourse.mybir as mybir
from concourse import bass_utils
from concourse.tile import TileContext

F32 = mybir.dt.float32
BF16 = mybir.dt.bfloat16
AF = mybir.ActivationFunctionType
ALU = mybir.AluOpType

B = 8
N_FULL = 4096
HALF_FULL = 2048
P = 128
R = 16              # argmin group size (candidates per gather)
KSPLIT = 21         # bf16 split-contraction terms
NEG_BIG = -3.0e38


def _build_nc(N=N_FULL, HALF=HALF_FULL, num_devices=B):
    NCH = N // P         # 128-row chunks per direction
    MMFD = min(512, HALF)
    NH = N // HALF       # psum tiles per chunk
    NG = N // R          # groups per row
    GH = HALF // R       # groups per psum tile

    nc = bacc.Bacc("TRN2", target_bir_lowering=False,
                   num_devices=num_devices)

    # augs columns: [x1_lhsT | y1_rhs | y2_lhsT | x2_rhs], each N wide, bf16.
    augs = nc.dram_tensor("augs", [KSPLIT, 4 * N], BF16,
                          kind="ExternalInput").ap()
    xr = nc.dram_tensor("xr", [N, 3], F32, kind="ExternalInput").ap()
    yr = nc.dram_tensor("yr", [N, 3], F32, kind="ExternalInput").ap()
    # consts row: [iota16 | iota16 + R]
    consts = nc.dram_tensor("consts", [P, 2 * R], F32,
                            kind="ExternalInput").ap()
    out_s = nc.dram_tensor("out_s", [P, 4], F32, kind="ExternalOutput").ap()

    with TileContext(nc) as tc:
        with (
            tc.tile_pool(name="const", bufs=1) as const_pool,
            tc.tile_pool(name="u", bufs=3) as u_pool,
            tc.tile_pool(name="r8", bufs=3) as r8_pool,
            tc.tile_pool(name="sgn", bufs=3) as sgn_pool,
            tc.tile_pool(name="idx", bufs=1) as idx_pool,
            tc.tile_pool(name="epi", bufs=1) as epi_pool,
            tc.tile_pool(name="psum", bufs=2, space="PSUM") as psum_pool,
        ):
            augs_sb = const_pool.tile([KSPLIT, 4 * N], BF16, tag="augs")
            # load order: dir-1 lhsT, then dir-1 rhs in quarters (so the
            # first matmuls start as soon as the first quarter lands),
            # then everything else.
            # lhsT on the SP ring, rhs quarters on the ACT ring — the
            # two HWDGE rings transfer in parallel, halving time-to-first-MM
            nc.sync.dma_start(augs_sb[:, 0:N], augs[:, 0:N])
            Q = N // 4
            for q in range(4):
                nc.scalar.dma_start(augs_sb[:, N + q * Q:N + (q + 1) * Q],
                                    augs[:, N + q * Q:N + (q + 1) * Q])

            def aug(i):
                return augs_sb[:, i * N:(i + 1) * N]

            consts_sb = const_pool.tile([P, 2 * R], F32, tag="consts")
            nc.sync.dma_start(consts_sb[:], consts)

            dummy = const_pool.tile([P, 1], F32, tag="dummy")
            nc.vector.memset(dummy[:], 0.0)

            # epilogue "own point" tiles — load up front, they only
            # depend on the raw inputs
            ow_t = {}
            for dirn, own in ((1, xr), (2, yr)):
                ow = epi_pool.tile([P, NCH, 3], F32, tag=f"ow{dirn}",
                                   name=f"ow{dirn}")
                nc.sync.dma_start(
                    ow[:], own.rearrange("(c p) d -> p c d", p=P))
                ow_t[dirn] = ow

            for i in (2, 3):
                nc.sync.dma_start(augs_sb[:, i * N:(i + 1) * N],
                                  augs[:, i * N:(i + 1) * N])

            idxg_f = {1: idx_pool.tile([P, NCH], F32, tag="ig1", name="ig1"),
                      2: idx_pool.tile([P, NCH], F32, tag="ig2", name="ig2")}
            idxg_i = {1: idx_pool.tile([P, NCH], mybir.dt.int32, tag="ii1",
                                       name="ii1"),
                      2: idx_pool.tile([P, NCH], mybir.dt.int32, tag="ii2",
                                       name="ii2")}
            # gathered candidate groups, flat [P, NCH * R * 3]
            cand = {1: epi_pool.tile([P, NCH * R * 3], F32, tag="cand1",
                                     name="cand1"),
                    2: epi_pool.tile([P, NCH * R * 3], F32, tag="cand2",
                                     name="cand2")}

            partials = epi_pool.tile([P, 4], F32, tag="partials")
            FC = NCH * R * 3   # candidate floats per partition
            FK = NCH * R       # candidates per partition

            def epilogue(dirn, half, c0, c1):
                """Exact within-group argmin + sum |diff|^5 for chunk range
                [c0, c1) of a direction; writes partials column
                (dirn-1)*2 + half."""
                NC_h = c1 - c0
                FCh = NC_h * R * 3
                FKh = NC_h * R
                hh = f"{dirn}_{half}"
                cd = cand[dirn][:, c0 * R * 3:c1 * R * 3]
                ow = ow_t[dirn]
                owb = bass.AP(ow[:].tensor, ow[:].offset + c0 * 3,
                              [ow[:].ap[0], [3, NC_h], [0, R], [1, 3]])

                diff = epi_pool.tile([P, FCh], F32, tag=f"df{hh}",
                                     name=f"df{hh}")
                nc.vector.tensor_sub(
                    diff[:].rearrange("p (c k d) -> p c k d", k=R, d=3),
                    owb, cd.rearrange("p (c k d) -> p c k d", k=R, d=3))
                # sq first (feeds the DVE distance chain); abs afterwards
                # (only feeds the |.|^5 path, which has slack)
                sq = epi_pool.tile([P, FCh], F32, tag=f"sq{hh}",
                                   name=f"sq{hh}")
                nc.scalar.activation(out=sq[:], in_=diff[:], func=AF.Square,
                                     bias=0.0, scale=1.0)
                ad = epi_pool.tile([P, FCh], F32, tag=f"ab{hh}",
                                   name=f"ab{hh}")
                nc.scalar.activation(out=ad[:], in_=diff[:], func=AF.Abs,
                                     bias=0.0, scale=1.0)
                # start the p5 chain early — it runs on ACT/GpSimd and
                # must not trail the DVE selection chain
                q4 = epi_pool.tile([P, FCh], F32, tag=f"q4{hh}",
                                   name=f"q4{hh}")
                nc.scalar.activation(out=q4[:], in_=sq[:], func=AF.Square,
                                     bias=0.0, scale=1.0)
                p5e = epi_pool.tile([P, FCh], F32, tag=f"p5{hh}",
                                    name=f"p5{hh}")
                nc.gpsimd.tensor_mul(p5e[:], q4[:], ad[:])
                # squared L2 distance per candidate
                dd = epi_pool.tile([P, FKh], F32, tag=f"dd{hh}",
                                   name=f"dd{hh}")
                nc.vector.tensor_reduce(
                    out=dd[:], in_=sq[:].rearrange("p (k d) -> p k d", d=3),
                    axis=mybir.AxisListType.X, op=ALU.add)
                # min distance per row
                dmin = epi_pool.tile([P, NC_h], F32, tag=f"dm{hh}",
                                     name=f"dm{hh}")
                nc.vector.tensor_reduce(
                    out=dmin[:], in_=dd[:].rearrange("p (c k) -> p c k", k=R),
                    axis=mybir.AxisListType.X, op=ALU.min)
                dminb = bass.AP(dmin[:].tensor, dmin[:].offset,
                                [dmin[:].ap[0], [1, NC_h], [0, R]])
                mask = epi_pool.tile([P, FKh], F32, tag=f"mk{hh}",
                                     name=f"mk{hh}")
                nc.vector.tensor_tensor(
                    out=mask[:].rearrange("p (c k) -> p c k", k=R),
                    in0=dd[:].rearrange("p (c k) -> p c k", k=R),
                    in1=dminb, op=ALU.is_le)
                # first-attaining candidate: k* = min_k (iota_k + R*(1-mask))
                iotap = bass.AP(consts_sb[:].tensor, consts_sb[:].offset + R,
                               [consts_sb[:].ap[0], [0, NC_h], [1, R]])
                tkm = epi_pool.tile([P, FKh], F32, tag=f"tm{hh}",
                                    name=f"tm{hh}")
                nc.vector.tensor_scalar_mul(tkm[:], mask[:], -float(R))
                tk = epi_pool.tile([P, FKh], F32, tag=f"tk{hh}",
                                   name=f"tk{hh}")
                nc.vector.tensor_tensor(
                    out=tk[:].rearrange("p (c k) -> p c k", k=R),
                    in0=tkm[:].rearrange("p (c k) -> p c k", k=R),
                    in1=iotap, op=ALU.add)
                kstar = epi_pool.tile([P, NC_h], F32, tag=f"ks{hh}",
                                      name=f"ks{hh}")
                nc.vector.tensor_reduce(
                    out=kstar[:], in_=tk[:].rearrange("p (c k) -> p c k", k=R),
                    axis=mybir.AxisListType.X, op=ALU.min)
                # tk = (iota+R) - R*mask: attaining entries are exactly
                # iota_k, others iota_k+R, so min over k is k*.
                ksb = bass.AP(kstar[:].tensor, kstar[:].offset,
                              [kstar[:].ap[0], [1, NC_h], [0, R]])
                onehot = epi_pool.tile([P, FKh], F32, tag=f"oh{hh}",
                                       name=f"oh{hh}")
                nc.vector.tensor_tensor(
                    out=onehot[:].rearrange("p (c k) -> p c k", k=R),
                    in0=bass.AP(consts_sb[:].tensor, consts_sb[:].offset,
                                [consts_sb[:].ap[0], [0, NC_h], [1, R]]),
                    in1=ksb, op=ALU.is_equal)
                p5k = epi_pool.tile([P, FKh], F32, tag=f"pk{hh}",
                                    name=f"pk{hh}")
                nc.vector.tensor_reduce(
                    out=p5k[:], in_=p5e[:].rearrange("p (k d) -> p k d", d=3),
                    axis=mybir.AxisListType.X, op=ALU.add)
                psel = epi_pool.tile([P, FKh], F32, tag=f"pl{hh}",
                                     name=f"pl{hh}")
                nc.gpsimd.tensor_mul(psel[:], p5k[:], onehot[:])
                col = (dirn - 1) * 2 + half
                nc.vector.reduce_sum(partials[:, col:col + 1], psel[:],
                                     axis=mybir.AxisListType.X)

            for dirn in (1, 2):
                lhsT_all = aug(0) if dirn == 1 else aug(2)
                rhs_all = aug(1) if dirn == 1 else aug(3)
                gsrc = yr if dirn == 1 else xr
                gsrc_g = gsrc.rearrange("(g k) d -> g (k d)", k=R)
                for c in range(NCH):
                    r8 = r8_pool.tile([P, NG], F32, tag="r8")
                    u = u_pool.tile([P, NG], F32, tag="u")
                    for h in range(NH):
                        ps = psum_pool.tile([P, HALF], F32, tag="ps",
                                            space="PSUM")
                        for k in range(HALF // MMFD):
                            m0 = h * HALF + k * MMFD
                            nc.tensor.matmul(
                                ps[:, k * MMFD:(k + 1) * MMFD],
                                lhsT=lhsT_all[:, c * P:(c + 1) * P],
                                rhs=rhs_all[:, m0:m0 + MMFD],
                                start=True, stop=True,
                            )
                        # group maxima (R-wide) of this tile, 1 elem/cyc
                        nc.vector.tensor_reduce(
                            out=u[:, h * GH:(h + 1) * GH],
                            in_=ps[:].rearrange("p (g k) -> p g k", k=R),
                            axis=mybir.AxisListType.X,
                            op=ALU.max,
                        )
                    # prefix max over groups (2 cyc/elem, N/R elems)
                    nc.vector.tensor_tensor_scan(
                        out=r8[:],
                        data0=u[:],
                        data1=dummy[:, 0:1].to_broadcast([P, NG]),
                        initial=NEG_BIG,
                        op0=ALU.max,
                        op1=ALU.bypass,
                    )
                    # group index of first attainment of the row max
                    sgn = sgn_pool.tile([P, NG], BF16, tag="sgn")
                    nc.scalar.activation(
                        out=sgn[:], in_=r8[:, :],
                        func=AF.Sign,
                        bias=r8[:, NG - 1:NG],
                        scale=-1.0,
                        accum_out=idxg_f[dirn][:, c:c + 1],
                    )
                    nc.scalar.activation(
                        out=idxg_i[dirn][:, c:c + 1],
                        in_=idxg_f[dirn][:, c:c + 1],
                        func=AF.Copy, bias=0.0, scale=1.0)
                    # gather the 16-candidate group (48 contiguous floats)
                    nc.gpsimd.indirect_dma_start(
                        out=cand[dirn][:, c * R * 3:(c + 1) * R * 3],
                        out_offset=None,
                        in_=gsrc_g,
                        in_offset=bass.IndirectOffsetOnAxis(
                            ap=idxg_i[dirn][:, c:c + 1], axis=0),
                    )
                    # first-half epilogue overlaps the remaining chunks;
                    # only the second half tails the direction
                    if c == min(NCH // 2 + 1, NCH - 2):
                        # shortly after the half point: the ACT/DMA deps
                        # have finished by the time the DVE's strict-FIFO
                        # queue reaches these ops
                        epilogue(dirn, 0, 0, NCH // 2)
                    elif c == NCH - 1:
                        epilogue(dirn, 1, NCH // 2, NCH)

            nc.sync.dma_start(out_s, partials[:])

    nc.compile()
    return nc


def _to_bf16(a):
    return a.astype(ml_dtypes.bfloat16)


def _split3(a):
    a = np.asarray(a, np.float32)
    h = _to_bf16(a)
    m = _to_bf16(a - h.astype(np.float32))
    l = _to_bf16(a - h.astype(np.float32) - m.astype(np.float32))
    return h, m, l


def _host_prep(xb, yb):
    xb = np.ascontiguousarray(xb, dtype=np.float32)
    yb = np.ascontiguousarray(yb, dtype=np.float32)
    n = xb.shape[0]
    ones = np.ones((n,), np.float32)

    def build(sta, mov, key_sq):
        """bf16 split terms for s = sum_c sta_c * (2 mov_c) - |mov|^2
        as seen with `sta` stationary; key_sq = -(|mov|^2)."""
        ta, tb = [], []
        for c in range(3):
            a, b = _split3(sta[:, c]), _split3(2.0 * mov[:, c])
            for i, j in ((0, 0), (0, 1), (0, 2), (1, 0), (1, 1), (2, 0)):
                ta.append(a[i])
                tb.append(b[j])
        sh, sm, sl = _split3(key_sq)
        ob = _to_bf16(ones)
        for s in (sh, sm, sl):
            ta.append(ob)
            tb.append(s)
        A = np.stack(ta).astype(ml_dtypes.bfloat16)
        Bm = np.stack(tb).astype(ml_dtypes.bfloat16)
        return A, Bm

    y2 = -(yb * yb).sum(-1)
    x2 = -(xb * xb).sum(-1)
    A1, B1 = build(xb, yb, y2)   # dir 1: lhsT = x terms, rhs = y terms
    A2, B2 = build(yb, xb, x2)   # dir 2: lhsT = y terms, rhs = x terms

    augs = np.empty((KSPLIT, 4 * n), ml_dtypes.bfloat16)
    augs[:, 0 * n:1 * n] = A1
    augs[:, 1 * n:2 * n] = B1
    augs[:, 2 * n:3 * n] = A2
    augs[:, 3 * n:4 * n] = B2

    iota = np.arange(R, dtype=np.float32)
    consts = np.tile(np.concatenate([iota, iota + R])[None, :], (P, 1))
    return {"augs": augs, "xr": xb, "yr": yb,
            "consts": np.ascontiguousarray(consts, np.float32)}


_NC = None


def _get_nc():
    global _NC
    if _NC is None:
        _NC = _build_nc()
    return _NC


def run_on_hw(x, y, **spmd_kwargs):
    """Run the SPMD kernel; returns (per-core out arrays, BassKernelResults)."""
    x = np.asarray(x, dtype=np.float32)
    y = np.asarray(y, dtype=np.float32)
    assert x.shape == (B, N_FULL, 3) and y.shape == (B, N_FULL, 3)
    nc = _get_nc()
    in_maps = [_host_prep(x[b], y[b]) for b in range(B)]
    res = bass_utils.run_bass_kernel_spmd(
        nc, in_maps, core_ids=list(range(B)), **spmd_kwargs)
    outs = [res.results[b]["out_s"] for b in range(B)]
    return outs, res


def kernel(x, y):
    outs, _ = run_on_hw(x, y)
    vals = []
    for o in outs:
        s = np.asarray(o, dtype=np.float64).sum(axis=0)  # [4] half-partials
        s1 = s[0] + s[1]
        s2 = s[2] + s[3]
        vals.append(s1 ** 0.2 + s2 ** 0.2)
    return np.float32(np.mean(vals))
